# revision 28
# baseline (speedup 1.0000x reference)
"""Trainium2 kernel for nn_MaskedRead (masked cross-attention read).

Reference computation (per batch b):
    logits = mk^T qk / sqrt(Dk)          [Nm, Nq]
    logits[~mm] = -1e30
    p      = softmax_m(logits)
    read   = mv @ p                      [Dv, Nq]
    out    = qv + (read where qm valid else 0)

Shapes: B=4, Dk=128, Dv=512, Nq=4096 (TQ*H*W), Nm=8192 (TM*H*W).

Strategy:
  * 8-way shard: data parallel over B=4, x2 split of the query axis.
    Host packs only the mask-valid query/memory positions (~50% Bernoulli
    masks) and ships packed bf16/fp8 pre-transposed operands.
  * Device per core:
      S[m_pair, q] = mk_t^T @ qk          (TensorE bf16, N=512)
      p = exp(S - 3) -> fp8e5m2           (ScalarE, one ACTIVATE per
          DoubleRow pair over 2 PSUM banks; e5m2 max 57344 >> e^8.3 so the
          logit tail cannot overflow -- TRN e4m3 maxes at 240 and NaNs
          above 256, which is why e4m3 p fails)
      readT & Z via fp8 DoubleRowSwInterleave (K=256): lhsT = p-pair
          [128, 128q, 2j] interleaved, rhs = mv-pair [128,2,*] e4m3 whose
          leading "ones" column is 1 only for valid m rows -- accumulator
          column 0 is then exactly Z and zero-padded phantom rows drop out
          of numerator AND denominator. Split 257+256 across two PSUM
          banks (512 fp32/bank).
      out = readT * (1/Z)                 (VectorE), bf16 -> HBM
  * Pipeline: a uniform stream of phases, one per (pass, chunk). Each
    phase runs phase1 (S matmuls + ACT-paced exp, PSUM double-buffered so
    PE and ScalarE overlap) while the PREVIOUS phase's PV matmuls fill
    the PE between groups. Tile's For_i cannot carry read-before-write
    tags across iterations, so the timing build unrolls UNROLL passes per
    body with pass-parity p/input buffers; fill/drain bubbles amortize.
  * Host adds qv in fp32 and scatters rows back to valid positions.
"""

import math

import numpy as np
import ml_dtypes

import concourse.mybir as mybir
import concourse.tile as tile
from concourse import bacc
from concourse.bass_utils import run_bass_kernel_spmd

B, DK, DV = 4, 128, 512
NQ_FULL = 4096
NM_FULL = 8192
N_CORES = 8
NQ_P = 1024            # queries per core (2 chunks x 512); overflow -> host
QCH = 512              # q-chunk width
DVP = DV + 16          # mv row: [ones | 512 data | 15 junk] (stride % 16 == 0)
PBIAS = -3.0           # constant logit shift; cancels in the softmax division
NEG = -1e30
UNROLL = 8             # passes per For_i body in the timing build

BF16 = mybir.dt.bfloat16
F32 = mybir.dt.float32
FP8E4 = mybir.dt.float8e4
FP8E5 = mybir.dt.float8e5
DRS = mybir.MatmulPerfMode.DoubleRowSwInterleave
EXPF = mybir.ActivationFunctionType.Exp

_NC_CACHE = {}


def build_nc(NMT, passes=1):
    """Build + compile the SPMD program for NMT m-tiles (NMT even).

    passes=1 emits one pass (the deliverable); passes=UNROLL*k wraps an
    UNROLL-pass body in a hardware For_i loop for steady-state timing."""
    key = (NMT, passes)
    if key in _NC_CACHE:
        return _NC_CACHE[key]
    assert NMT % 2 == 0
    NM_P = NMT * 128
    NU = NMT // 2          # DoubleRow pairs
    half_t = NMT // 2      # m-tiles per mk half-DMA

    # split the mv DMA into 2 spans so early PV pairs start sooner
    mv_spans = []
    u0 = 0
    nsp = 2
    for si in range(nsp):
        n = (NU - u0 + (nsp - si) - 1) // (nsp - si)
        if n <= 0:
            break
        mv_spans.append((u0, u0 + n))
        u0 += n

    nc = bacc.Bacc("TRN2", target_bir_lowering=False, debug=False,
                   num_devices=N_CORES)
    qk_d = nc.dram_tensor("qk", [DK, NQ_P], BF16, kind="ExternalInput")
    mk_d = nc.dram_tensor("mk", [DK, NM_P], BF16, kind="ExternalInput")
    mv_d = nc.dram_tensor("mv", [128, NU, 2, DVP], FP8E4, kind="ExternalInput")
    out_d = nc.dram_tensor("readT", [NQ_P, DV], BF16, kind="ExternalOutput")

    with tile.TileContext(nc) as tc:
        with (
            tc.tile_pool(name="consts", bufs=1) as consts,
            tc.tile_pool(name="inp", bufs=1) as inp,
            tc.tile_pool(name="pp", bufs=1) as pp,
            tc.tile_pool(name="spsum", bufs=2, space="PSUM") as spsum,
            tc.tile_pool(name="rpsum", bufs=1, space="PSUM") as rpsum,
            tc.tile_pool(name="outp", bufs=2) as outp,
            tc.tile_pool(name="small", bufs=3) as small,
        ):
            bias_sb = consts.tile([128, 1], F32, name="bias_sb")
            nc.vector.memset(bias_sb, PBIAS)

            def emit_body(npass):
                """npass passes; pass k uses input/p buffers of parity
                k % 2. Each phase's PV work fills the next phase's
                ACT-paced exp stream; the last phase's PV drains at the
                tail."""

                inputs = {}    # parity -> (qkc, mk_halves, mv_spans)

                def dma_inputs(par):
                    qkc = []
                    for c in range(2):
                        t = inp.tile([128, QCH], BF16, tag=f"qk{par}c{c}",
                                     name=f"qk{par}c{c}")
                        nc.sync.dma_start(
                            out=t, in_=qk_d[:, c * QCH:(c + 1) * QCH])
                        qkc.append(t)
                    mkh = []
                    for h in range(2):
                        t = inp.tile([128, half_t * 128], BF16,
                                     tag=f"mk{par}h{h}", name=f"mk{par}h{h}")
                        nc.sync.dma_start(
                            out=t, in_=mk_d[:, h * half_t * 128:
                                            (h + 1) * half_t * 128])
                        mkh.append(t)
                    mvs = []
                    for si, (a, b) in enumerate(mv_spans):
                        t = inp.tile([128, b - a, 2, DVP], FP8E4,
                                     tag=f"mv{par}s{si}",
                                     name=f"mv{par}s{si}")
                        nc.sync.dma_start(out=t, in_=mv_d[:, a:b, :, :])
                        mvs.append((a, b, t))
                    inputs[par] = (qkc, mkh, mvs)

                def mk_sl(par, t):
                    h, tt = divmod(t, half_t)
                    return inputs[par][1][h][:, tt * 128:(tt + 1) * 128]

                def mv_sl(par, u):
                    for (a, b, t) in inputs[par][2]:
                        if a <= u < b:
                            return t[:, u - a]
                    raise AssertionError(u)

                acc = {}

                def pv_pair(key, qt, u, pt, par, tags, first, last):
                    """One DoubleRow pair-step of the PV accumulation:
                    readT into (ra cols 1..256) ++ (rb cols 0..255), and Z
                    into ra col 0 via the mv ones column."""
                    if first:
                        acc[(key, qt)] = (
                            rpsum.tile([128, 512], F32, tag=tags[0],
                                       name=tags[0]),
                            rpsum.tile([128, 512], F32, tag=tags[1],
                                       name=tags[1]))
                    ra, rb = acc[(key, qt)]
                    lhsT = pt[:, qt]              # [128, 128q, 2j]
                    rhs = mv_sl(par, u)
                    nc.tensor.matmul(
                        ra[:, 0:257], lhsT=lhsT, rhs=rhs[:, :, 0:257],
                        start=first, stop=last,
                        perf_mode=DRS, skip_group_check=True)
                    nc.tensor.matmul(
                        rb[:, 0:256], lhsT=lhsT, rhs=rhs[:, :, 257:513],
                        start=first, stop=last,
                        perf_mode=DRS, skip_group_check=True)

                omega = {}

                def epilogue(key, c, qt):
                    ra, rb = acc.pop((key, qt))
                    rz = small.tile([128, 1], F32, tag="rz", name="rz")
                    nc.vector.reciprocal(rz, ra[:, 0:1])
                    o = omega[key]
                    qt_abs = c * 4 + qt
                    nc.vector.tensor_scalar_mul(o[:, qt_abs, 0:256],
                                                ra[:, 1:257], rz)
                    nc.vector.tensor_scalar_mul(o[:, qt_abs, 256:512],
                                                rb[:, 0:256], rz)

                def pv_units(key, c, par, pts):
                    """All PV for one (pass, chunk): four q-tile streaks,
                    alternating between two accumulator-bank sets. The
                    chunk's half of a per-pass [128, 8, DV] staging tile is
                    filled; chunk 1 ends with one 1MB output DMA."""
                    us = []

                    def open_stage():
                        omega[key] = omega.get((key[0],)) or outp.tile(
                            [128, 8, DV], BF16, tag="o", name="o")
                        omega[(key[0],)] = omega[key]
                    if c == 0:
                        us.append(open_stage)
                    else:
                        omega[key] = omega[(key[0],)]
                    for qt in range(4):
                        tags = (("raL0", "rbL0") if qt % 2 == 0
                                else ("raL1", "rbL1"))
                        for u in range(NU):
                            us.append(
                                lambda key=key, qt=qt, u=u, pts=pts,
                                par=par, tags=tags: pv_pair(
                                    key, qt, u, pts[u], par, tags,
                                    u == 0, u == NU - 1))
                        us.append(lambda key=key, c=c, qt=qt: epilogue(
                            key, c, qt))
                    if c == 1:
                        def flush(key=key):
                            nc.sync.dma_start(
                                out=out_d.rearrange("(a p) v -> p a v",
                                                    p=128),
                                in_=omega.pop(key))
                            omega.pop((key[0],), None)
                        us.append(flush)
                    return us

                def phase1(c, par, pkey, fill):
                    """Per pair u: 2 S matmuls + 1 batched exp (fp8e5m2,
                    SwInterleaved layout); `fill` (previous phase's PV)
                    spread between the ACT-paced groups."""
                    pts = []
                    nf = len(fill)
                    taken = 0
                    for u in range(NU):
                        s = spsum.tile([128, 2, QCH], F32, tag="s",
                                       name="s")
                        for j in (0, 1):
                            nc.tensor.matmul(
                                s[:, j], lhsT=mk_sl(par, 2 * u + j),
                                rhs=inputs[par][0][c],
                                start=True, stop=True)
                        pt = pp.tile([128, 4, 128, 2], FP8E5,
                                     tag=f"p{pkey}u{u}", name=f"p{pkey}u{u}")
                        nc.scalar.activation(
                            out=pt.rearrange("p a q j -> p j a q"),
                            in_=s.rearrange("p j (a q) -> p j a q", a=4),
                            func=EXPF, bias=bias_sb[:, 0:1], scale=1.0)
                        pts.append(pt)
                        upto = (nf * (u + 1)) // NU
                        while taken < upto:
                            fill[taken]()
                            taken += 1
                    while taken < nf:
                        fill[taken]()
                        taken += 1
                    return pts

                pending = []
                for k in range(npass):
                    par = k % 2
                    dma_inputs(par)
                    for c in range(2):
                        pkey = f"{c}{par}"
                        pts = phase1(c, par, pkey, pending)
                        pending = pv_units((k, c), c, par, pts)
                for f in pending:
                    f()

            if passes == 1:
                emit_body(1)
            else:
                assert passes % UNROLL == 0
                with tc.For_i(0, passes // UNROLL, 1,
                              hint_engines=(mybir.EngineType.PE,
                                            mybir.EngineType.Activation,
                                            mybir.EngineType.DVE,
                                            mybir.EngineType.SP,
                                            mybir.EngineType.Pool)):
                    emit_body(UNROLL)

    nc.compile()
    _NC_CACHE[key] = nc
    return nc


def _ceilmul(n, m):
    return max(m, ((n + m - 1) // m) * m)


def prepare(qkey, qval, qmask, mkey, mval, mmask):
    """Shard + pack the full inputs. Returns (in_maps, meta)."""
    qk = np.asarray(qkey, dtype=np.float32).reshape(B, DK, NQ_FULL)
    qv = np.asarray(qval, dtype=np.float32).reshape(B, DV, NQ_FULL)
    qm = np.asarray(qmask).reshape(B, NQ_FULL).astype(bool)
    mk = np.asarray(mkey, dtype=np.float32).reshape(B, DK, NM_FULL)
    mv = np.asarray(mval, dtype=np.float32).reshape(B, DV, NM_FULL)
    mm = np.asarray(mmask).reshape(B, NM_FULL).astype(bool)

    scale = 1.0 / math.sqrt(DK)
    shards = []          # per core: (b, qidx_shard, valid)
    leftovers = []       # (b, qidx_overflow) handled on host
    midx_b, valid_b = [], []
    for b in range(B):
        qidx = np.nonzero(qm[b])[0]
        midx = np.nonzero(mm[b])[0]
        valid = (qidx.size > 0) and (midx.size > 0)
        midx_b.append(midx)
        valid_b.append(valid)
        shards.append((b, qidx[:NQ_P], valid))
        shards.append((b, qidx[NQ_P:2 * NQ_P], valid))
        if valid and qidx.size > 2 * NQ_P:
            leftovers.append((b, qidx[2 * NQ_P:]))

    NM_P = max(_ceilmul(mi.size, 256) for mi in midx_b)
    NMT = NM_P // 128
    NU = NMT // 2

    # per-batch packed mk / mv (shared by both shards of a batch)
    mk_b, mv_b = {}, {}
    for b in range(B):
        mi = midx_b[b]
        a_mk = np.zeros((DK, NM_P), dtype=ml_dtypes.bfloat16)
        a_mv = np.zeros((128, NU, 2, DVP), dtype=ml_dtypes.float8_e4m3fn)
        if valid_b[b]:
            a_mk[:, :mi.size] = mk[b][:, mi].astype(ml_dtypes.bfloat16)
            full = np.zeros((NM_P, DVP), dtype=np.float32)
            full[:mi.size, 0] = 1.0          # ones col -> Z; 0 on padding
            full[:mi.size, 1:1 + DV] = np.clip(mv[b][:, mi].T, -240, 240)
            a_mv = np.ascontiguousarray(
                full.reshape(NU, 2, 128, DVP).transpose(2, 0, 1, 3)
            ).astype(ml_dtypes.float8_e4m3fn)
        mk_b[b] = a_mk
        mv_b[b] = a_mv

    in_maps = []
    for (b, qi, valid) in shards:
        a_qk = np.zeros((DK, NQ_P), dtype=ml_dtypes.bfloat16)
        if valid and qi.size:
            a_qk[:, :qi.size] = (qk[b][:, qi] * scale).astype(
                ml_dtypes.bfloat16)
        in_maps.append({"qk": a_qk, "mk": mk_b[b], "mv": mv_b[b]})

    # Host-side exact fp32 attention for overflow query columns (rare)
    host_cols = []
    for (b, qi) in leftovers:
        mi = midx_b[b]
        s = mk[b][:, mi].T @ (qk[b][:, qi] * scale)
        s -= s.max(axis=0, keepdims=True)
        p = np.exp(s)
        p /= p.sum(axis=0, keepdims=True)
        host_cols.append((b, qi, mv[b][:, mi] @ p))

    meta = dict(qv=qv, shards=shards, NMT=NMT,
                host_cols=host_cols, out_shape=np.asarray(qval).shape)
    return in_maps, meta


def finish(results, meta):
    out = meta["qv"].copy()
    for core, (b, qi, valid) in enumerate(meta["shards"]):
        if not valid or qi.size == 0:
            continue
        readT = np.asarray(results[core]["readT"], dtype=np.float32)
        # SwInterleave's reversed-column convention with forward storage
        # makes each 128-row block come back q-reversed
        readT = readT.reshape(NQ_P // 128, 128, DV)[:, ::-1].reshape(
            NQ_P, DV)
        readT = readT[:qi.size]
        out[b][:, qi] += readT.T
    for (b, qi, read_cols) in meta["host_cols"]:
        out[b][:, qi] += read_cols
    return out.reshape(meta["out_shape"]).astype(np.float32)


def kernel(qkey, qval, qmask, mkey, mval, mmask):
    in_maps, meta = prepare(qkey, qval, qmask, mkey, mval, mmask)
    nc = build_nc(meta["NMT"])
    res = run_bass_kernel_spmd(nc, in_maps, core_ids=list(range(N_CORES)))
    return finish(res.results, meta)


def hw_time_ns(in_maps, meta, p_lo=UNROLL, p_hi=40 * UNROLL * 250, reps=5):
    """Differential wall-clock estimate of per-pass HW time.

    The axon/PJRT proxy adds a large jittery constant per execute; compare
    min wall-clock of a p_hi-pass build vs a p_lo-pass build (interleaved
    sampling) to cancel it. Returns (ns_per_pass, details)."""
    import time as _time
    ncs = {p: build_nc(meta["NMT"], passes=p) for p in (p_lo, p_hi)}
    ts = {p: [] for p in (p_lo, p_hi)}
    for _ in range(reps):
        for p in (p_lo, p_hi):
            t0 = _time.perf_counter()
            run_bass_kernel_spmd(ncs[p], in_maps,
                                 core_ids=list(range(N_CORES)))
            ts[p].append(_time.perf_counter() - t0)
    ns = (min(ts[p_hi]) - min(ts[p_lo])) / (p_hi - p_lo) * 1e9
    return ns, {p: min(v) for p, v in ts.items()}


# revision 31
# speedup vs baseline: 1.0229x; 1.0229x over previous
"""Trainium2 kernel for nn_MaskedRead (masked cross-attention read).

Reference computation (per batch b):
    logits = mk^T qk / sqrt(Dk)          [Nm, Nq]
    logits[~mm] = -1e30
    p      = softmax_m(logits)
    read   = mv @ p                      [Dv, Nq]
    out    = qv + (read where qm valid else 0)

Shapes: B=4, Dk=128, Dv=512, Nq=4096 (TQ*H*W), Nm=8192 (TM*H*W).

Strategy:
  * 8-way shard: data parallel over B=4, x2 split of the query axis.
    Host packs only the mask-valid query/memory positions (~50% Bernoulli
    masks) and ships packed bf16/fp8 pre-transposed operands.
  * Device per core:
      S[m_pair, q] = mk_t^T @ qk          (TensorE bf16, N=512)
      p = exp(S - 3) -> fp8e5m2           (ScalarE, one ACTIVATE per
          DoubleRow pair over 2 PSUM banks; e5m2 max 57344 >> e^8.3 so the
          logit tail cannot overflow -- TRN e4m3 maxes at 240 and NaNs
          above 256, which is why e4m3 p fails)
      readT & Z via fp8 DoubleRowSwInterleave (K=256): lhsT = p-pair
          [128, 128q, 2j] interleaved, rhs = mv-pair [128,2,*] e4m3 whose
          leading "ones" column is 1 only for valid m rows -- accumulator
          column 0 is then exactly Z and zero-padded phantom rows drop out
          of numerator AND denominator. Split 257+256 across two PSUM
          banks (512 fp32/bank).
      out = readT * (1/Z)                 (VectorE), bf16 -> HBM
  * Pipeline: a uniform stream of phases, one per (pass, chunk). Each
    phase runs phase1 (S matmuls + ACT-paced exp, PSUM double-buffered so
    PE and ScalarE overlap) while the PREVIOUS phase's PV matmuls fill
    the PE between groups. Tile's For_i cannot carry read-before-write
    tags across iterations, so the timing build unrolls UNROLL passes per
    body with pass-parity p/input buffers; fill/drain bubbles amortize.
  * Host adds qv in fp32 and scatters rows back to valid positions.
"""

import math

import numpy as np
import ml_dtypes

import concourse.mybir as mybir
import concourse.tile as tile
from concourse import bacc
from concourse.bass_utils import run_bass_kernel_spmd

B, DK, DV = 4, 128, 512
NQ_FULL = 4096
NM_FULL = 8192
N_CORES = 8
NQ_P = 1024            # queries per core (2 chunks x 512); overflow -> host
QCH = 512              # q-chunk width
DVP = DV + 16          # mv row: [ones | 512 data | 15 junk] (stride % 16 == 0)
PBIAS = -3.0           # constant logit shift; cancels in the softmax division
NEG = -1e30
UNROLL = 8             # passes per For_i body in the timing build

BF16 = mybir.dt.bfloat16
F32 = mybir.dt.float32
FP8E4 = mybir.dt.float8e4
FP8E5 = mybir.dt.float8e5
DRS = mybir.MatmulPerfMode.DoubleRowSwInterleave
EXPF = mybir.ActivationFunctionType.Exp

_NC_CACHE = {}


def build_nc(NMT, passes=1):
    """Build + compile the SPMD program for NMT m-tiles (NMT even).

    passes=1 emits one pass (the deliverable); passes=UNROLL*k wraps an
    UNROLL-pass body in a hardware For_i loop for steady-state timing."""
    key = (NMT, passes)
    if key in _NC_CACHE:
        return _NC_CACHE[key]
    assert NMT % 2 == 0
    NM_P = NMT * 128
    NU = NMT // 2          # DoubleRow pairs
    half_t = NMT // 2      # m-tiles per mk half-DMA

    # split the mv DMA into 2 spans so early PV pairs start sooner
    mv_spans = []
    u0 = 0
    nsp = 2
    for si in range(nsp):
        n = (NU - u0 + (nsp - si) - 1) // (nsp - si)
        if n <= 0:
            break
        mv_spans.append((u0, u0 + n))
        u0 += n

    nc = bacc.Bacc("TRN2", target_bir_lowering=False, debug=False,
                   num_devices=N_CORES)
    qk_d = nc.dram_tensor("qk", [DK, NQ_P], BF16, kind="ExternalInput")
    mk_d = nc.dram_tensor("mk", [DK, NM_P], BF16, kind="ExternalInput")
    mv_d = nc.dram_tensor("mv", [128, NU, 2, DVP], FP8E4, kind="ExternalInput")
    out_d = nc.dram_tensor("readT", [NQ_P, DV], BF16, kind="ExternalOutput")

    with tile.TileContext(nc) as tc:
        with (
            tc.tile_pool(name="consts", bufs=1) as consts,
            tc.tile_pool(name="inp", bufs=1) as inp,
            tc.tile_pool(name="pp", bufs=1) as pp,
            tc.tile_pool(name="spsum", bufs=2, space="PSUM") as spsum,
            tc.tile_pool(name="rpsum", bufs=1, space="PSUM") as rpsum,
            tc.tile_pool(name="outp", bufs=3) as outp,
            tc.tile_pool(name="small", bufs=3) as small,
        ):
            bias_sb = consts.tile([128, 1], F32, name="bias_sb")
            nc.vector.memset(bias_sb, PBIAS)

            def emit_body(npass):
                """npass passes; pass k uses input/p buffers of parity
                k % 2. Each phase's PV work fills the next phase's
                ACT-paced exp stream; the last phase's PV drains at the
                tail."""

                inputs = {}    # parity -> (qkc, mk_halves, mv_spans)

                def dma_inputs(par):
                    qkc = []
                    for c in range(2):
                        t = inp.tile([128, QCH], BF16, tag=f"qk{par}c{c}",
                                     name=f"qk{par}c{c}")
                        nc.sync.dma_start(
                            out=t, in_=qk_d[:, c * QCH:(c + 1) * QCH])
                        qkc.append(t)
                    mkh = []
                    for h in range(2):
                        t = inp.tile([128, half_t * 128], BF16,
                                     tag=f"mk{par}h{h}", name=f"mk{par}h{h}")
                        nc.sync.dma_start(
                            out=t, in_=mk_d[:, h * half_t * 128:
                                            (h + 1) * half_t * 128])
                        mkh.append(t)
                    mvs = []
                    for si, (a, b) in enumerate(mv_spans):
                        t = inp.tile([128, b - a, 2, DVP], FP8E4,
                                     tag=f"mv{par}s{si}",
                                     name=f"mv{par}s{si}")
                        nc.sync.dma_start(out=t, in_=mv_d[:, a:b, :, :])
                        mvs.append((a, b, t))
                    inputs[par] = (qkc, mkh, mvs)

                def mk_sl(par, t):
                    h, tt = divmod(t, half_t)
                    return inputs[par][1][h][:, tt * 128:(tt + 1) * 128]

                def mv_sl(par, u):
                    for (a, b, t) in inputs[par][2]:
                        if a <= u < b:
                            return t[:, u - a]
                    raise AssertionError(u)

                acc = {}

                def pv_pair(key, qt, u, pt, par, tags, first, last):
                    """One DoubleRow pair-step of the PV accumulation:
                    readT into (ra cols 1..256) ++ (rb cols 0..255), and Z
                    into ra col 0 via the mv ones column."""
                    if first:
                        acc[(key, qt)] = (
                            rpsum.tile([128, 512], F32, tag=tags[0],
                                       name=tags[0]),
                            rpsum.tile([128, 512], F32, tag=tags[1],
                                       name=tags[1]))
                    ra, rb = acc[(key, qt)]
                    lhsT = pt[:, qt]              # [128, 128q, 2j]
                    rhs = mv_sl(par, u)
                    nc.tensor.matmul(
                        ra[:, 0:257], lhsT=lhsT, rhs=rhs[:, :, 0:257],
                        start=first, stop=last,
                        perf_mode=DRS, skip_group_check=True)
                    nc.tensor.matmul(
                        rb[:, 0:256], lhsT=lhsT, rhs=rhs[:, :, 257:513],
                        start=first, stop=last,
                        perf_mode=DRS, skip_group_check=True)

                def epilogue(key, c, qt):
                    ra, rb = acc.pop((key, qt))
                    rz = small.tile([128, 1], F32, tag="rz", name="rz")
                    nc.vector.reciprocal(rz, ra[:, 0:1])
                    o = outp.tile([128, DV], BF16, tag="o", name="o")
                    nc.vector.tensor_scalar_mul(o[:, 0:256],
                                                ra[:, 1:257], rz)
                    nc.vector.tensor_scalar_mul(o[:, 256:512],
                                                rb[:, 0:256], rz)
                    qt_abs = c * 4 + qt
                    nc.sync.dma_start(
                        out=out_d[qt_abs * 128:(qt_abs + 1) * 128, :], in_=o)

                def pv_units(key, c, par, pts):
                    """All PV for one (pass, chunk): four q-tile streaks,
                    alternating between two accumulator-bank sets."""
                    us = []
                    for qt in range(4):
                        tags = (("raL0", "rbL0") if qt % 2 == 0
                                else ("raL1", "rbL1"))
                        for u in range(NU):
                            us.append(
                                lambda key=key, qt=qt, u=u, pts=pts,
                                par=par, tags=tags: pv_pair(
                                    key, qt, u, pts[u], par, tags,
                                    u == 0, u == NU - 1))
                        us.append(lambda key=key, c=c, qt=qt: epilogue(
                            key, c, qt))
                    return us

                def phase1(c, par, pkey, fill):
                    """Per pair u: 2 S matmuls + 1 batched exp (fp8e5m2,
                    SwInterleaved layout); `fill` (previous phase's PV)
                    spread between the ACT-paced groups."""
                    pts = []
                    nf = len(fill)
                    taken = 0
                    for u in range(NU):
                        s = spsum.tile([128, 2, QCH], F32, tag="s",
                                       name="s")
                        for j in (0, 1):
                            nc.tensor.matmul(
                                s[:, j], lhsT=mk_sl(par, 2 * u + j),
                                rhs=inputs[par][0][c],
                                start=True, stop=True)
                        pt = pp.tile([128, 4, 128, 2], FP8E5,
                                     tag=f"p{pkey}u{u}", name=f"p{pkey}u{u}")
                        nc.scalar.activation(
                            out=pt.rearrange("p a q j -> p j a q"),
                            in_=s.rearrange("p j (a q) -> p j a q", a=4),
                            func=EXPF, bias=bias_sb[:, 0:1], scale=1.0)
                        pts.append(pt)
                        upto = (nf * (u + 1)) // NU
                        while taken < upto:
                            fill[taken]()
                            taken += 1
                    while taken < nf:
                        fill[taken]()
                        taken += 1
                    return pts

                pending = []
                for k in range(npass):
                    par = k % 2
                    dma_inputs(par)
                    for c in range(2):
                        pkey = f"{c}{par}"
                        pts = phase1(c, par, pkey, pending)
                        pending = pv_units((k, c), c, par, pts)
                for f in pending:
                    f()

            if passes == 1:
                emit_body(1)
            else:
                assert passes % UNROLL == 0
                with tc.For_i(0, passes // UNROLL, 1,
                              hint_engines=(mybir.EngineType.PE,
                                            mybir.EngineType.Activation,
                                            mybir.EngineType.DVE,
                                            mybir.EngineType.SP,
                                            mybir.EngineType.Pool)):
                    emit_body(UNROLL)

    nc.compile()
    _NC_CACHE[key] = nc
    return nc


def _ceilmul(n, m):
    return max(m, ((n + m - 1) // m) * m)


def prepare(qkey, qval, qmask, mkey, mval, mmask):
    """Shard + pack the full inputs. Returns (in_maps, meta)."""
    qk = np.asarray(qkey, dtype=np.float32).reshape(B, DK, NQ_FULL)
    qv = np.asarray(qval, dtype=np.float32).reshape(B, DV, NQ_FULL)
    qm = np.asarray(qmask).reshape(B, NQ_FULL).astype(bool)
    mk = np.asarray(mkey, dtype=np.float32).reshape(B, DK, NM_FULL)
    mv = np.asarray(mval, dtype=np.float32).reshape(B, DV, NM_FULL)
    mm = np.asarray(mmask).reshape(B, NM_FULL).astype(bool)

    scale = 1.0 / math.sqrt(DK)
    shards = []          # per core: (b, qidx_shard, valid)
    leftovers = []       # (b, qidx_overflow) handled on host
    midx_b, valid_b = [], []
    for b in range(B):
        qidx = np.nonzero(qm[b])[0]
        midx = np.nonzero(mm[b])[0]
        valid = (qidx.size > 0) and (midx.size > 0)
        midx_b.append(midx)
        valid_b.append(valid)
        shards.append((b, qidx[:NQ_P], valid))
        shards.append((b, qidx[NQ_P:2 * NQ_P], valid))
        if valid and qidx.size > 2 * NQ_P:
            leftovers.append((b, qidx[2 * NQ_P:]))

    NM_P = max(_ceilmul(mi.size, 256) for mi in midx_b)
    NMT = NM_P // 128
    NU = NMT // 2

    # per-batch packed mk / mv (shared by both shards of a batch)
    mk_b, mv_b = {}, {}
    for b in range(B):
        mi = midx_b[b]
        a_mk = np.zeros((DK, NM_P), dtype=ml_dtypes.bfloat16)
        a_mv = np.zeros((128, NU, 2, DVP), dtype=ml_dtypes.float8_e4m3fn)
        if valid_b[b]:
            a_mk[:, :mi.size] = mk[b][:, mi].astype(ml_dtypes.bfloat16)
            full = np.zeros((NM_P, DVP), dtype=np.float32)
            full[:mi.size, 0] = 1.0          # ones col -> Z; 0 on padding
            full[:mi.size, 1:1 + DV] = np.clip(mv[b][:, mi].T, -240, 240)
            a_mv = np.ascontiguousarray(
                full.reshape(NU, 2, 128, DVP).transpose(2, 0, 1, 3)
            ).astype(ml_dtypes.float8_e4m3fn)
        mk_b[b] = a_mk
        mv_b[b] = a_mv

    in_maps = []
    for (b, qi, valid) in shards:
        a_qk = np.zeros((DK, NQ_P), dtype=ml_dtypes.bfloat16)
        if valid and qi.size:
            a_qk[:, :qi.size] = (qk[b][:, qi] * scale).astype(
                ml_dtypes.bfloat16)
        in_maps.append({"qk": a_qk, "mk": mk_b[b], "mv": mv_b[b]})

    # Host-side exact fp32 attention for overflow query columns (rare)
    host_cols = []
    for (b, qi) in leftovers:
        mi = midx_b[b]
        s = mk[b][:, mi].T @ (qk[b][:, qi] * scale)
        s -= s.max(axis=0, keepdims=True)
        p = np.exp(s)
        p /= p.sum(axis=0, keepdims=True)
        host_cols.append((b, qi, mv[b][:, mi] @ p))

    meta = dict(qv=qv, shards=shards, NMT=NMT,
                host_cols=host_cols, out_shape=np.asarray(qval).shape)
    return in_maps, meta


def finish(results, meta):
    out = meta["qv"].copy()
    for core, (b, qi, valid) in enumerate(meta["shards"]):
        if not valid or qi.size == 0:
            continue
        readT = np.asarray(results[core]["readT"], dtype=np.float32)
        # SwInterleave's reversed-column convention with forward storage
        # makes each 128-row block come back q-reversed
        readT = readT.reshape(NQ_P // 128, 128, DV)[:, ::-1].reshape(
            NQ_P, DV)
        readT = readT[:qi.size]
        out[b][:, qi] += readT.T
    for (b, qi, read_cols) in meta["host_cols"]:
        out[b][:, qi] += read_cols
    return out.reshape(meta["out_shape"]).astype(np.float32)


def kernel(qkey, qval, qmask, mkey, mval, mmask):
    in_maps, meta = prepare(qkey, qval, qmask, mkey, mval, mmask)
    nc = build_nc(meta["NMT"])
    res = run_bass_kernel_spmd(nc, in_maps, core_ids=list(range(N_CORES)))
    return finish(res.results, meta)


def hw_time_ns(in_maps, meta, p_lo=UNROLL, p_hi=40 * UNROLL * 250, reps=7):
    """Differential wall-clock estimate of per-pass HW time.

    The axon/PJRT proxy adds a large jittery constant per execute; compare
    min wall-clock of a p_hi-pass build vs a p_lo-pass build (interleaved
    sampling) to cancel it. Returns (ns_per_pass, details)."""
    import time as _time
    ncs = {p: build_nc(meta["NMT"], passes=p) for p in (p_lo, p_hi)}
    ts = {p: [] for p in (p_lo, p_hi)}
    for _ in range(reps):
        for p in (p_lo, p_hi):
            t0 = _time.perf_counter()
            run_bass_kernel_spmd(ncs[p], in_maps,
                                 core_ids=list(range(N_CORES)))
            ts[p].append(_time.perf_counter() - t0)
    ns = (min(ts[p_hi]) - min(ts[p_lo])) / (p_hi - p_lo) * 1e9
    return ns, {p: min(v) for p, v in ts.items()}
